# revision 3
# baseline (speedup 1.0000x reference)
"""GAE Trainium2 kernel, PE-matvec variant.

Problem: nn_CustomGAE — B=512, T=2048, D=64; pure data parallel over B
across 8 cores (64 trajectories/core).

Design: the value-head matvecs (obs@W, next_obs@W) run on the TENSOR
engine as fp32r matmuls (1 cycle/row at >=256 out cols), leaving the DVE
with only the td0/scan/target epilogue. Measured pure-IO floor is ~157us
(438 GB/s/core) while the old all-DVE kernel ran ~204-227us, so freeing
the DVE from the streaming matvec is the whole game.

Matmul structure (per stream unit = 32 trajectories x one 256-step
time-block q): rhs rows r = dd*32 + j32 pack 32 trajectories x 4
features; 16 accumulating matmuls (one per d-slice ds, d = ds*4 + dd)
against W_stat_ds[r, m] = W[ds*4+dd] * (j32 == m%32) produce
psum[128, 256] where partition m holds trajectory m%32's value,
replicated 4x. A plain lane-aligned engine tensor_copy then evicts the
[32, 256] window for partitions s*64+32*g2.. straight into scan layout
(PSUM cannot be a DMA source here, and engine copies cannot shift
partitions — replication makes every 32-aligned window correct).

Layouts (per core, host pre-swizzled):
- obs/nobs DRAM [128, 65536]: row r = dd*32 + j32; col =
  U*4096 + ds*256 + t_l with unit U = (7-q)*2 + g2 (time-blocks q
  streamed LATEST-FIRST so the reverse GAE scan runs incrementally);
  trajectory = 32*g2 + j32, t = q*256 + t_l.
- reward/done/adv/tgt DRAM [128, 1024] interleaved scan layout:
  partition p = s*64 + j (s = 1 - q%2), col = qr2*256 + t_l
  (qr2 = (7-q)//2). Alternating q between partition groups makes each
  q-block complete as soon as its 2 units land; the scan chain
  (q=7 -> 0, seeded across groups by a [64,1] boundary DMA) overlaps
  the stream and only the last q-block's epilogue is tail (~2us).

Engines: SP ring = input stream; PE = matmuls; Act = obs evictions +
g/coef factors; Pool = done->f32; DVE = nobs evictions + epilogue;
output/boundary DMAs also on the Act ring (DVE cannot issue DMAs).
"""

import sys

sys.path.insert(0, "/opt/trn_rl_repo")

from contextlib import ExitStack

import numpy as np

import concourse.bacc as bacc
import concourse.mybir as mybir
import concourse.tile as tile
from concourse.bass_utils import run_bass_kernel_spmd

GAMMA = 0.99
LMBDA = 0.95

B, T, D = 512, 2048, 64
NCORES = 8
BL = B // NCORES  # 64 trajectories per core
NQ = 4  # time blocks
NDS = 16  # d-slices per matvec (D // 4)
F32 = mybir.dt.float32
F32R = mybir.dt.float32r
U8 = mybir.dt.uint8

LAST_RESULTS = None


def _sq(q):
    """Partition group s for time-block q."""
    return 1 - (q % 2)


def _qr2(q, nq=NQ):
    """Column block (of nq//2) for time-block q within its partition group."""
    return (nq - 1 - q) // 2


def build_program(
    t_total=T, repeat=1, bench_internal=False, bufs=8, psum_bufs=4, nq=NQ,
    evict="act", out_ring="gpsimd", nocompute=False, wm=128,
):
    tq = t_total // nq          # timesteps per block (256 at nq=8)
    tp = t_total // 2           # 1024 cols per partition in scan layout
    ucols = NDS * tq            # cols per stream unit

    nc = bacc.Bacc(
        "TRN2", target_bir_lowering=False, debug=False, enable_asserts=False
    )

    big_kind = "Internal" if bench_internal else "ExternalInput"
    obs_d = nc.dram_tensor("obs", [128, 2 * nq * ucols], F32R, kind=big_kind)
    nobs_d = nc.dram_tensor("nobs", [128, 2 * nq * ucols], F32R, kind=big_kind)
    rw_d = nc.dram_tensor("rw", [128, tp], F32, kind="ExternalInput")
    dn_d = nc.dram_tensor("dn", [128, tp], U8, kind="ExternalInput")
    wst_d = nc.dram_tensor("wst", [128, NDS * wm], F32R, kind="ExternalInput")
    b_d = nc.dram_tensor("b", [1], F32, kind="ExternalInput")
    adv_d = nc.dram_tensor("adv", [128, tp], F32, kind="ExternalOutput")
    tgt_d = nc.dram_tensor("tgt", [128, tp], F32, kind="ExternalOutput")

    mult = mybir.AluOpType.mult
    add = mybir.AluOpType.add
    sub = mybir.AluOpType.subtract

    with tile.TileContext(nc) as tc, ExitStack() as ctx:
        cpool = ctx.enter_context(tc.tile_pool(name="const", bufs=1))
        pers = ctx.enter_context(tc.tile_pool(name="pers", bufs=1))
        spool = ctx.enter_context(tc.tile_pool(name="stream", bufs=bufs))
        ppool = ctx.enter_context(
            tc.tile_pool(name="ps", space="PSUM", bufs=psum_bufs)
        )

        wst_t = cpool.tile([128, NDS * wm], F32R)
        nc.scalar.dma_start(wst_t[:], wst_d.ap())
        b_t = cpool.tile([128, 1], F32)
        nc.scalar.dma_start(b_t[:], b_d.ap().unsqueeze(0).broadcast_to([128, 1]))
        bnd = cpool.tile([128, 1], F32)

        rw_t = pers.tile([128, tp], F32)
        nc.scalar.dma_start(rw_t[:], rw_d.ap())
        dn_t = pers.tile([128, tp], U8)
        nc.scalar.dma_start(dn_t[:], dn_d.ap())

        ndf = pers.tile([128, tp], F32)
        g = pers.tile([128, tp], F32)
        coef = pers.tile([128, tp], F32)
        v_raw = pers.tile([128, tp], F32)
        nv_raw = pers.tile([128, tp], F32)
        td0 = pers.tile([128, tp], F32)
        sc1 = pers.tile([128, tp], F32)
        sc2 = pers.tile([128, tp], F32)
        adv = pers.tile([128, tp], F32)
        tgt = pers.tile([128, tp], F32)

        # done -> gamma*nd and gamma*lambda*nd (Pool converts, Act scales)
        nc.gpsimd.tensor_copy(ndf[:], dn_t[:])
        nc.scalar.activation(
            g[:], ndf[:], mybir.ActivationFunctionType.Copy,
            bias=GAMMA, scale=-GAMMA,
        )
        nc.scalar.activation(
            coef[:], ndf[:], mybir.ActivationFunctionType.Copy,
            bias=GAMMA * LMBDA, scale=-GAMMA * LMBDA,
        )

        for _rep in range(repeat):
            for u in range(2 * nq):
                q = nq - 1 - u // 2
                g2 = u % 2
                s = _sq(q)
                colq = _qr2(q, nq) * tq
                rs32 = slice(s * 64 + 32 * g2, s * 64 + 32 * g2 + 32)
                ev2 = "scalar" if evict == "act" else "vector"
                nhalf = max(1, ucols * 4 // 16384)  # 16KB stream tiles
                hcols = ucols // nhalf
                hds = NDS // nhalf
                for tens_d, vdst, ev in (
                    (obs_d, v_raw, "scalar"),
                    (nobs_d, nv_raw, ev2),
                ):
                    ps = None if nocompute else ppool.tile([128, tq], F32)
                    for hf in range(nhalf):
                        st = spool.tile([128, hcols], F32R)
                        nc.sync.dma_start(
                            st[:],
                            tens_d.ap()[
                                :, u * ucols + hf * hcols:
                                u * ucols + (hf + 1) * hcols
                            ],
                        )
                        if nocompute:
                            continue
                        mm_out = ps[:] if wm == 128 else ps[rs32, :]
                        tpos = None if wm == 128 else (0, rs32.start % 128)
                        for dsl in range(hds):
                            ds = hf * hds + dsl
                            nc.tensor.matmul(
                                out=mm_out,
                                lhsT=wst_t[:, ds * wm:(ds + 1) * wm],
                                rhs=st[:, dsl * tq:(dsl + 1) * tq],
                                start=(ds == 0),
                                stop=(ds == NDS - 1),
                                tile_position=tpos,
                            )
                    if nocompute:
                        continue
                    if ev == "scalar":
                        nc.scalar.copy(vdst[rs32, colq:colq + tq], ps[rs32, :])
                    else:
                        nc.vector.tensor_copy(
                            vdst[rs32, colq:colq + tq], ps[rs32, :]
                        )
                # after the nobs unit of g2==1, block q is complete
                if g2 == 1 and nocompute:
                    oring = nc.gpsimd if out_ring == "gpsimd" else nc.scalar
                    rs = slice(s * 64, s * 64 + 64)
                    cs = slice(colq, colq + tq)
                    oring.dma_start(adv_d.ap()[rs, cs], rw_t[rs, cs])
                    oring.dma_start(tgt_d.ap()[rs, cs], rw_t[rs, cs])
                elif g2 == 1:
                    rs = slice(s * 64, s * 64 + 64)
                    cs = slice(colq, colq + tq)
                    bsc = b_t[rs, 0:1]
                    # sc1 = (nv + b) * g ; sc2 = (v + b) - rw
                    nc.vector.scalar_tensor_tensor(
                        out=sc1[rs, cs], in0=nv_raw[rs, cs], scalar=bsc,
                        in1=g[rs, cs], op0=add, op1=mult,
                    )
                    nc.vector.scalar_tensor_tensor(
                        out=sc2[rs, cs], in0=v_raw[rs, cs], scalar=bsc,
                        in1=rw_t[rs, cs], op0=add, op1=sub,
                    )
                    # td0 = sc1 - sc2 = rw + g*(nv+b) - (v+b)
                    nc.vector.tensor_tensor(
                        out=td0[rs, cs], in0=sc1[rs, cs], in1=sc2[rs, cs],
                        op=sub,
                    )
                    if q == nq - 1:
                        init = 0.0
                    else:
                        init = bnd[rs, 0:1]
                    nc.vector.tensor_tensor_scan(
                        out=adv[rs, cs][:, ::-1],
                        data0=coef[rs, cs][:, ::-1],
                        data1=td0[rs, cs][:, ::-1],
                        initial=init,
                        op0=mult,
                        op1=add,
                    )
                    # boundary seed for the next block (other group)
                    oring = nc.gpsimd if out_ring == "gpsimd" else nc.scalar
                    if q > 0:
                        ns = _sq(q - 1)
                        oring.dma_start(
                            bnd[ns * 64:ns * 64 + 64, 0:1],
                            adv[rs, colq:colq + 1],
                        )
                    # tgt = (adv + b) + v
                    nc.vector.scalar_tensor_tensor(
                        out=tgt[rs, cs], in0=adv[rs, cs], scalar=bsc,
                        in1=v_raw[rs, cs], op0=add, op1=add,
                    )
                    oring.dma_start(adv_d.ap()[rs, cs], adv[rs, cs])
                    oring.dma_start(tgt_d.ap()[rs, cs], tgt[rs, cs])

    nc.finalize()
    return nc


_NC_CACHE = None


def _get_nc():
    global _NC_CACHE
    if _NC_CACHE is None:
        _NC_CACHE = build_program()
    return _NC_CACHE


def _pe_swizzle(x, t_total, nq=NQ):
    """[BL, T, D] -> [128, 2*nq*NDS*(T//nq)] PE stream layout (see header)."""
    tq = t_total // nq
    # dims (g2, j32, q, t_l, ds, dd); row = dd*32 + j32
    x = x.reshape(2, 32, nq, tq, NDS, 4)
    # -> (dd, j32, q, g2, ds, t_l), stream q latest-first
    x = x.transpose(5, 1, 2, 0, 4, 3)[:, :, ::-1]
    return np.ascontiguousarray(x).reshape(128, 2 * nq * NDS * tq)


def _scan_swizzle(x, t_total, dtype, nq=NQ):
    """[BL, T] -> [128, T//2] interleaved scan layout."""
    tq = t_total // nq
    y = np.empty((2, BL, nq // 2, tq), dtype=dtype)
    xr = x.reshape(BL, nq, tq)
    for q in range(nq):
        y[_sq(q), :, _qr2(q, nq), :] = xr[:, q, :]
    return y.reshape(128, t_total // 2)


def _scan_unswizzle(y, t_total, nq=NQ):
    """Inverse of _scan_swizzle: [128, T//2] -> [BL, T]."""
    tq = t_total // nq
    yr = y.reshape(2, BL, nq // 2, tq)
    x = np.empty((BL, nq, tq), dtype=y.dtype)
    for q in range(nq):
        x[:, q, :] = yr[_sq(q), :, _qr2(q, nq), :]
    return x.reshape(BL, t_total)


def make_wst(W, wm=128):
    """W_stat [128, NDS*wm]: wst[r, ds*wm+m] = W[ds*4 + r//32] if
    r%32 == m%32 else 0 (wm=32 drops the replicated column groups; the
    matmul then writes psum partitions [base, base+32) directly)."""
    w_np = np.asarray(W, dtype=np.float32).reshape(D)
    r = np.arange(128)
    m = np.arange(wm)
    sel = (r[:, None] % 32) == (m[None, :] % 32)  # [128, wm]
    wst = np.zeros((128, NDS * wm), np.float32)
    for ds in range(NDS):
        blk = sel * w_np[ds * 4 + r // 32][:, None]
        wst[:, ds * wm:(ds + 1) * wm] = blk
    return wst


def shard_inputs(obs, next_obs, reward, done, W, b, nq=NQ, wm=128):
    obs = np.asarray(obs, dtype=np.float32).reshape(B, T, D)
    nobs = np.asarray(next_obs, dtype=np.float32).reshape(B, T, D)
    rw = np.asarray(reward, dtype=np.float32).reshape(B, T)
    dn = np.asarray(done).astype(np.uint8, copy=False).reshape(B, T)
    wst = make_wst(W, wm)
    b_np = np.ascontiguousarray(np.asarray(b, dtype=np.float32)).reshape(1)

    in_maps = []
    for i in range(NCORES):
        sl = slice(i * BL, (i + 1) * BL)
        in_maps.append(
            {
                "obs": _pe_swizzle(obs[sl], T, nq),
                "nobs": _pe_swizzle(nobs[sl], T, nq),
                "rw": _scan_swizzle(rw[sl], T, np.float32, nq),
                "dn": _scan_swizzle(dn[sl], T, np.uint8, nq),
                "wst": wst,
                "b": b_np,
            }
        )
    return in_maps


def gather_outputs(results, nq=NQ):
    advantage = np.concatenate(
        [_scan_unswizzle(r["adv"], T, nq) for r in results], axis=0
    ).reshape(B, T, 1)
    value_target = np.concatenate(
        [_scan_unswizzle(r["tgt"], T, nq) for r in results], axis=0
    ).reshape(B, T, 1)
    return advantage, value_target


def kernel(obs, next_obs, reward, done, W, b):
    global LAST_RESULTS
    nc = _get_nc()
    in_maps = shard_inputs(obs, next_obs, reward, done, W, b)
    res = run_bass_kernel_spmd(nc, in_maps, core_ids=list(range(NCORES)))
    LAST_RESULTS = res
    return gather_outputs(res.results)


# revision 4
# speedup vs baseline: 1.1295x; 1.1295x over previous
"""GAE Trainium2 kernel, PE-matvec variant.

Problem: nn_CustomGAE — B=512, T=2048, D=64; pure data parallel over B
across 8 cores (64 trajectories/core).

Design: the value-head matvecs (obs@W, next_obs@W) run on the TENSOR
engine as fp32r matmuls (1 cycle/row at >=256 out cols), leaving the DVE
with only the td0/scan/target epilogue. Measured pure-IO floor is ~157us
(438 GB/s/core) while the old all-DVE kernel ran ~204-227us, so freeing
the DVE from the streaming matvec is the whole game.

Matmul structure (per stream unit = 32 trajectories x one 256-step
time-block q): rhs rows r = dd*32 + j32 pack 32 trajectories x 4
features; 16 accumulating matmuls (one per d-slice ds, d = ds*4 + dd)
against W_stat_ds[r, m] = W[ds*4+dd] * (j32 == m%32) produce
psum[128, 256] where partition m holds trajectory m%32's value,
replicated 4x. A plain lane-aligned engine tensor_copy then evicts the
[32, 256] window for partitions s*64+32*g2.. straight into scan layout
(PSUM cannot be a DMA source here, and engine copies cannot shift
partitions — replication makes every 32-aligned window correct).

Layouts (per core, host pre-swizzled):
- obs/nobs DRAM [128, 65536]: row r = dd*32 + j32; col =
  U*4096 + ds*256 + t_l with unit U = (7-q)*2 + g2 (time-blocks q
  streamed LATEST-FIRST so the reverse GAE scan runs incrementally);
  trajectory = 32*g2 + j32, t = q*256 + t_l.
- reward/done/adv/tgt DRAM [128, 1024] interleaved scan layout:
  partition p = s*64 + j (s = 1 - q%2), col = qr2*256 + t_l
  (qr2 = (7-q)//2). Alternating q between partition groups makes each
  q-block complete as soon as its 2 units land; the scan chain
  (q=7 -> 0, seeded across groups by a [64,1] boundary DMA) overlaps
  the stream and only the last q-block's epilogue is tail (~2us).

Engines: SP ring = input stream; PE = matmuls; Act = obs evictions +
g/coef factors; Pool = done->f32; DVE = nobs evictions + epilogue;
output/boundary DMAs also on the Act ring (DVE cannot issue DMAs).
"""

import sys

sys.path.insert(0, "/opt/trn_rl_repo")

from contextlib import ExitStack

import numpy as np

import concourse.bacc as bacc
import concourse.mybir as mybir
import concourse.tile as tile
from concourse.bass_utils import run_bass_kernel_spmd

GAMMA = 0.99
LMBDA = 0.95

B, T, D = 512, 2048, 64
NCORES = 8
BL = B // NCORES  # 64 trajectories per core
NQ = 4  # time blocks
NDS = 16  # d-slices per matvec (D // 4)
F32 = mybir.dt.float32
F32R = mybir.dt.float32r
U8 = mybir.dt.uint8

LAST_RESULTS = None


def _sq(q):
    """Partition group s for time-block q."""
    return 1 - (q % 2)


def _qr2(q, nq=NQ):
    """Column block (of nq//2) for time-block q within its partition group."""
    return (nq - 1 - q) // 2


def build_program(
    t_total=T, repeat=1, bench_internal=False, bufs=8, psum_bufs=4, nq=NQ,
    evict="act", out_ring="gpsimd", nocompute=False, wm=128, tile_kb=16,
):
    tq = t_total // nq          # timesteps per block (256 at nq=8)
    tp = t_total // 2           # 1024 cols per partition in scan layout
    ucols = NDS * tq            # cols per stream unit

    nc = bacc.Bacc(
        "TRN2", target_bir_lowering=False, debug=False, enable_asserts=False
    )

    big_kind = "Internal" if bench_internal else "ExternalInput"
    obs_d = nc.dram_tensor("obs", [128, 2 * nq * ucols], F32R, kind=big_kind)
    nobs_d = nc.dram_tensor("nobs", [128, 2 * nq * ucols], F32R, kind=big_kind)
    rw_d = nc.dram_tensor("rw", [128, tp], F32, kind="ExternalInput")
    dn_d = nc.dram_tensor("dn", [128, tp], U8, kind="ExternalInput")
    wst_d = nc.dram_tensor("wst", [128, NDS * wm], F32R, kind="ExternalInput")
    b_d = nc.dram_tensor("b", [1], F32, kind="ExternalInput")
    adv_d = nc.dram_tensor("adv", [128, tp], F32, kind="ExternalOutput")
    tgt_d = nc.dram_tensor("tgt", [128, tp], F32, kind="ExternalOutput")

    mult = mybir.AluOpType.mult
    add = mybir.AluOpType.add
    sub = mybir.AluOpType.subtract

    with tile.TileContext(nc) as tc, ExitStack() as ctx:
        cpool = ctx.enter_context(tc.tile_pool(name="const", bufs=1))
        pers = ctx.enter_context(tc.tile_pool(name="pers", bufs=1))
        spool = ctx.enter_context(tc.tile_pool(name="stream", bufs=bufs))
        ppool = ctx.enter_context(
            tc.tile_pool(name="ps", space="PSUM", bufs=psum_bufs)
        )

        wst_t = cpool.tile([128, NDS * wm], F32R)
        nc.scalar.dma_start(wst_t[:], wst_d.ap())
        b_t = cpool.tile([128, 1], F32)
        nc.scalar.dma_start(b_t[:], b_d.ap().unsqueeze(0).broadcast_to([128, 1]))
        bnd = cpool.tile([128, 1], F32)

        rw_t = pers.tile([128, tp], F32)
        nc.scalar.dma_start(rw_t[:], rw_d.ap())
        dn_t = pers.tile([128, tp], U8)
        nc.scalar.dma_start(dn_t[:], dn_d.ap())

        ndf = pers.tile([128, tp], F32)
        g = pers.tile([128, tp], F32)
        coef = pers.tile([128, tp], F32)
        v_raw = pers.tile([128, tp], F32)
        nv_raw = pers.tile([128, tp], F32)
        td0 = pers.tile([128, tp], F32)
        sc1 = pers.tile([128, tp], F32)
        sc2 = pers.tile([128, tp], F32)
        adv = pers.tile([128, tp], F32)
        tgt = pers.tile([128, tp], F32)

        # done -> gamma*nd and gamma*lambda*nd (Pool converts, Act scales)
        nc.gpsimd.tensor_copy(ndf[:], dn_t[:])
        nc.scalar.activation(
            g[:], ndf[:], mybir.ActivationFunctionType.Copy,
            bias=GAMMA, scale=-GAMMA,
        )
        nc.scalar.activation(
            coef[:], ndf[:], mybir.ActivationFunctionType.Copy,
            bias=GAMMA * LMBDA, scale=-GAMMA * LMBDA,
        )

        for _rep in range(repeat):
            for u in range(2 * nq):
                q = nq - 1 - u // 2
                g2 = u % 2
                s = _sq(q)
                colq = _qr2(q, nq) * tq
                rs32 = slice(s * 64 + 32 * g2, s * 64 + 32 * g2 + 32)
                ev2 = "scalar" if evict == "act" else "vector"
                nhalf = max(1, ucols * 4 // (tile_kb * 1024))
                hcols = ucols // nhalf
                hds = NDS // nhalf
                for tens_d, vdst, ev in (
                    (obs_d, v_raw, "scalar"),
                    (nobs_d, nv_raw, ev2),
                ):
                    ps = None if nocompute else ppool.tile([128, tq], F32)
                    for hf in range(nhalf):
                        st = spool.tile([128, hcols], F32R)
                        nc.sync.dma_start(
                            st[:],
                            tens_d.ap()[
                                :, u * ucols + hf * hcols:
                                u * ucols + (hf + 1) * hcols
                            ],
                        )
                        if nocompute:
                            continue
                        mm_out = ps[:] if wm == 128 else ps[rs32, :]
                        tpos = None if wm == 128 else (0, rs32.start % 128)
                        for dsl in range(hds):
                            ds = hf * hds + dsl
                            nc.tensor.matmul(
                                out=mm_out,
                                lhsT=wst_t[:, ds * wm:(ds + 1) * wm],
                                rhs=st[:, dsl * tq:(dsl + 1) * tq],
                                start=(ds == 0),
                                stop=(ds == NDS - 1),
                                tile_position=tpos,
                            )
                    if nocompute:
                        continue
                    if ev == "scalar":
                        nc.scalar.copy(vdst[rs32, colq:colq + tq], ps[rs32, :])
                    else:
                        nc.vector.tensor_copy(
                            vdst[rs32, colq:colq + tq], ps[rs32, :]
                        )
                # after the nobs unit of g2==1, block q is complete
                if g2 == 1 and nocompute:
                    oring = nc.gpsimd if out_ring == "gpsimd" else nc.scalar
                    rs = slice(s * 64, s * 64 + 64)
                    cs = slice(colq, colq + tq)
                    oring.dma_start(adv_d.ap()[rs, cs], rw_t[rs, cs])
                    oring.dma_start(tgt_d.ap()[rs, cs], rw_t[rs, cs])
                elif g2 == 1:
                    rs = slice(s * 64, s * 64 + 64)
                    cs = slice(colq, colq + tq)
                    bsc = b_t[rs, 0:1]
                    # sc1 = (nv + b) * g ; sc2 = (v + b) - rw
                    nc.vector.scalar_tensor_tensor(
                        out=sc1[rs, cs], in0=nv_raw[rs, cs], scalar=bsc,
                        in1=g[rs, cs], op0=add, op1=mult,
                    )
                    nc.vector.scalar_tensor_tensor(
                        out=sc2[rs, cs], in0=v_raw[rs, cs], scalar=bsc,
                        in1=rw_t[rs, cs], op0=add, op1=sub,
                    )
                    # td0 = sc1 - sc2 = rw + g*(nv+b) - (v+b)
                    nc.vector.tensor_tensor(
                        out=td0[rs, cs], in0=sc1[rs, cs], in1=sc2[rs, cs],
                        op=sub,
                    )
                    if q == nq - 1:
                        init = 0.0
                    else:
                        init = bnd[rs, 0:1]
                    nc.vector.tensor_tensor_scan(
                        out=adv[rs, cs][:, ::-1],
                        data0=coef[rs, cs][:, ::-1],
                        data1=td0[rs, cs][:, ::-1],
                        initial=init,
                        op0=mult,
                        op1=add,
                    )
                    # boundary seed for the next block (other group)
                    oring = nc.gpsimd if out_ring == "gpsimd" else nc.scalar
                    if q > 0:
                        ns = _sq(q - 1)
                        oring.dma_start(
                            bnd[ns * 64:ns * 64 + 64, 0:1],
                            adv[rs, colq:colq + 1],
                        )
                    # tgt = (adv + b) + v
                    nc.vector.scalar_tensor_tensor(
                        out=tgt[rs, cs], in0=adv[rs, cs], scalar=bsc,
                        in1=v_raw[rs, cs], op0=add, op1=add,
                    )
                    oring.dma_start(adv_d.ap()[rs, cs], adv[rs, cs])
                    oring.dma_start(tgt_d.ap()[rs, cs], tgt[rs, cs])

    nc.finalize()
    return nc


_NC_CACHE = None


def _get_nc():
    global _NC_CACHE
    if _NC_CACHE is None:
        _NC_CACHE = build_program()
    return _NC_CACHE


def _pe_swizzle(x, t_total, nq=NQ):
    """[BL, T, D] -> [128, 2*nq*NDS*(T//nq)] PE stream layout (see header)."""
    tq = t_total // nq
    # dims (g2, j32, q, t_l, ds, dd); row = dd*32 + j32
    x = x.reshape(2, 32, nq, tq, NDS, 4)
    # -> (dd, j32, q, g2, ds, t_l), stream q latest-first
    x = x.transpose(5, 1, 2, 0, 4, 3)[:, :, ::-1]
    return np.ascontiguousarray(x).reshape(128, 2 * nq * NDS * tq)


def _scan_swizzle(x, t_total, dtype, nq=NQ):
    """[BL, T] -> [128, T//2] interleaved scan layout."""
    tq = t_total // nq
    y = np.empty((2, BL, nq // 2, tq), dtype=dtype)
    xr = x.reshape(BL, nq, tq)
    for q in range(nq):
        y[_sq(q), :, _qr2(q, nq), :] = xr[:, q, :]
    return y.reshape(128, t_total // 2)


def _scan_unswizzle(y, t_total, nq=NQ):
    """Inverse of _scan_swizzle: [128, T//2] -> [BL, T]."""
    tq = t_total // nq
    yr = y.reshape(2, BL, nq // 2, tq)
    x = np.empty((BL, nq, tq), dtype=y.dtype)
    for q in range(nq):
        x[:, q, :] = yr[_sq(q), :, _qr2(q, nq), :]
    return x.reshape(BL, t_total)


def make_wst(W, wm=128):
    """W_stat [128, NDS*wm]: wst[r, ds*wm+m] = W[ds*4 + r//32] if
    r%32 == m%32 else 0 (wm=32 drops the replicated column groups; the
    matmul then writes psum partitions [base, base+32) directly)."""
    w_np = np.asarray(W, dtype=np.float32).reshape(D)
    r = np.arange(128)
    m = np.arange(wm)
    sel = (r[:, None] % 32) == (m[None, :] % 32)  # [128, wm]
    wst = np.zeros((128, NDS * wm), np.float32)
    for ds in range(NDS):
        blk = sel * w_np[ds * 4 + r // 32][:, None]
        wst[:, ds * wm:(ds + 1) * wm] = blk
    return wst


def shard_inputs(obs, next_obs, reward, done, W, b, nq=NQ, wm=128):
    obs = np.asarray(obs, dtype=np.float32).reshape(B, T, D)
    nobs = np.asarray(next_obs, dtype=np.float32).reshape(B, T, D)
    rw = np.asarray(reward, dtype=np.float32).reshape(B, T)
    dn = np.asarray(done).astype(np.uint8, copy=False).reshape(B, T)
    wst = make_wst(W, wm)
    b_np = np.ascontiguousarray(np.asarray(b, dtype=np.float32)).reshape(1)

    in_maps = []
    for i in range(NCORES):
        sl = slice(i * BL, (i + 1) * BL)
        in_maps.append(
            {
                "obs": _pe_swizzle(obs[sl], T, nq),
                "nobs": _pe_swizzle(nobs[sl], T, nq),
                "rw": _scan_swizzle(rw[sl], T, np.float32, nq),
                "dn": _scan_swizzle(dn[sl], T, np.uint8, nq),
                "wst": wst,
                "b": b_np,
            }
        )
    return in_maps


def gather_outputs(results, nq=NQ):
    advantage = np.concatenate(
        [_scan_unswizzle(r["adv"], T, nq) for r in results], axis=0
    ).reshape(B, T, 1)
    value_target = np.concatenate(
        [_scan_unswizzle(r["tgt"], T, nq) for r in results], axis=0
    ).reshape(B, T, 1)
    return advantage, value_target


def kernel(obs, next_obs, reward, done, W, b):
    global LAST_RESULTS
    nc = _get_nc()
    in_maps = shard_inputs(obs, next_obs, reward, done, W, b)
    res = run_bass_kernel_spmd(nc, in_maps, core_ids=list(range(NCORES)))
    LAST_RESULTS = res
    return gather_outputs(res.results)
